# revision 1
# baseline (speedup 1.0000x reference)
"""CatLayer Trainium2 kernel (fp16 inputs, fp32 accumulate/output).

Math: out[i,j,b,:] = W @ leaky_relu(concat(x[i,b,:], x[j,b,:])) + bias
Since leaky_relu is elementwise over the concat:
    y  = leaky_relu(x)                    # (l, b, d)
    A  = y @ W[:, :d].T + bias            # (l, b, d)   "xi half"
    B  = y @ W[:, d:].T                   # (l, b, d)   "xj half"
    out[i,j,b,:] = A[i,b,:] + B[j,b,:]

Sharding: i-rows of the (l x l) pair grid over 8 cores (12 rows each).
Every core computes B for all j from full x; A only for its own i rows.

The kernel is DMA-bound: the fp32 output (37.75 MB/core) alone costs
~105 us at the 360 GB/s aggregate DMA bandwidth, so everything else is
arranged to keep the DMA device saturated:
  * x and W are loaded as fp16 (half the input bytes); matmuls run
    fp16 x fp16 -> fp32 PSUM (rel err ~3e-4, far under the 2e-2 gate).
  * Each core's xT is packed host-side with its own 192 (i,b) columns
    rotated to the front, so the A path reads them straight out of the
    shared yT tile - no separate xiT input. The host un-rotates the
    j axis of the output shard with np.roll.
  * All input DMAs are emitted on the sync queue before any store, in
    dependency order: x head columns [0,256) per k slice (feeds the
    relus + A + first two B j-tiles), then W per-k slices, then the
    x tails. First stores become ready right as the input stream
    drains, so the DMA device never idles mid-kernel.

Inputs are packed host-side into SBUF layout so each is a fully
contiguous DMA (partition dim leading):
    xT   (128, KT*T) fp16:  xT[p, k*T + u] = x[(u + r*TI) % T, 128k+p]
    W_in (128, 8*D) fp16:   W_in[p, g*D+c] = W.T[128g+p, c] (g<4: W1)
    bias (1, D) fp32
    out  (12*l*b, d) fp32

Engines:
    ACT: Prelu(alpha=0.1) + B-path PSUM->SBUF drains
    PE : A/B matmuls (fp16), fp32 one-hot E-matmul for the 16->128
         partition broadcast of A[i]
    DVE: tensor_add for all output tiles + A-path PSUM drains
    DMA: big contiguous transfers; out stores are one per (i, j-group)
"""

import numpy as np
from contextlib import ExitStack

import concourse.bacc as bacc
import concourse.mybir as mybir
from concourse import tile
from concourse.bass_utils import run_bass_kernel_spmd

F32 = mybir.dt.float32
F16 = mybir.dt.float16
AF = mybir.ActivationFunctionType

L, Bdim, D = 96, 16, 512
NCORES = 8
LPC = L // NCORES          # 12 i-rows per core
T = L * Bdim               # 1536 (j,b) rows
NT = T // 128              # 12 j-tiles
KT = D // 128              # 4 k-tiles
TI = LPC * Bdim            # 192 own (i,b) rows
XH = 256                   # head columns per k slice (>= TI, 2 j-tiles)
NEG_SLOPE = 0.1


def build_nc(repeats: int = 1, group_sizes=(1, 1, 1, 1, 1, 2, 2, 3), gps_groups=()):
    """Build the per-core Bass program (identical on all cores)."""
    assert sum(group_sizes) == NT
    g_off = [0]
    for g in group_sizes:
        g_off.append(g_off[-1] + g)

    nc = bacc.Bacc("TRN2", target_bir_lowering=False, debug=False)

    xT = nc.dram_tensor("xT", (128, KT * T), F16, kind="ExternalInput")
    w_in = nc.dram_tensor("w_in", (128, 2 * KT * D), F16, kind="ExternalInput")
    bias = nc.dram_tensor("bias", (1, D), F16, kind="ExternalInput")
    out = nc.dram_tensor("out", (LPC * T, D), F32, kind="ExternalOutput")

    # One-hot E for the 16->128 partition broadcast of A rows, replicated
    # with period 32 down all 128 rows so any legal 32-aligned window has
    # identical content: ec[par][g, p] == 1 iff g % 32 == 16*par + p % 16
    ec_np = np.zeros((2, 128, 128), np.float16)
    for par in range(2):
        for g in range(128):
            for p in range(128):
                if g % 32 == 16 * par + (p % 16):
                    ec_np[par, g, p] = 1.0
    ec_dram = nc.inline_tensor(ec_np, "Ec")

    with tile.TileContext(nc) as tc, ExitStack() as ctx:
        persist = ctx.enter_context(tc.tile_pool(name="persist", bufs=1))
        stage = ctx.enter_context(tc.tile_pool(name="stage", bufs=2))
        # PSUM banks: A chain 1, B tiles 3, E-broadcast 4 (of 8 total)
        psum = ctx.enter_context(tc.tile_pool(name="psum", bufs=6, space="PSUM"))
        outp = ctx.enter_context(tc.tile_pool(name="outp", bufs=3))

        # ---- small constants (scalar queue so they don't block sync)
        bias_sb = persist.tile([1, D], F16, tag="bias", name="bias_sb")
        nc.scalar.dma_start(bias_sb[:], bias[:])
        ones_sb = persist.tile([1, D], F16, tag="ones", name="ones_sb")
        nc.vector.memset(ones_sb[:], 1.0)
        ec_all = persist.tile([128, 256], F16, tag="ec", name="ec_all")
        nc.scalar.dma_start(
            ec_all[:].rearrange("g (a p) -> g a p", a=2),
            ec_dram.ap().rearrange("a g p -> g a p"),
        )
        ec_sb = [ec_all[:, :128], ec_all[:, 128:]]
        w_sb = persist.tile([128, 2 * KT * D], F16, tag="w", name="w_sb")

        def w1s(k):
            return w_sb[:, k * D : (k + 1) * D]

        def w2s(k):
            return w_sb[:, (KT + k) * D : (KT + k + 1) * D]

        # ---- PE warm-up: HAM runs the PE at half clock until it has seen
        # ~3.4us of activity. Issue dummy matmuls (ones x ones) wide enough
        # to keep the array busy until the first real matmuls arrive with
        # W1 (~4.2us), so they run at full clock.
        # Many narrow warmups: each is only ~107ns, so the last one never
        # delays the first real matmul by much, while keeping the PE busy
        # (and ramping) until W1 lands at ~5.6us.
        warm_ps = psum.tile([128, D], F32, tag="eps", bufs=4, name="warm_ps")
        for _ in range(44):
            nc.tensor.matmul(
                warm_ps[:, :128], ones_sb[:1, :128], ones_sb[:1, :128],
                start=True, stop=True,
            )

        for rep in range(repeats):
            x_st = stage.tile(
                [128, KT * T], F16, tag="x_st", bufs=1, name=f"x_st_{rep}"
            )
            # ---- all input DMAs, in dependency order, before any store.
            # Four big transfers: each DMA has a ~650ns descriptor-gen pitch
            # through SEQ/HWDGE, so small per-k slices would starve the DMA
            # engines. Strided APs batch all k slices into one instruction.
            # Head: columns [0, XH) of each k slice (own TI block + first
            # two B j-tiles) so relu/A/B can start while W still streams.
            x_st_v = x_st[:].rearrange("p (k t) -> p k t", k=KT)
            xT_v = xT.ap().rearrange("p (k t) -> p k t", k=KT)
            nc.sync.dma_start(x_st_v[:, :, :XH], xT_v[:, :, :XH])
            if rep == 0:
                nc.sync.dma_start(w_sb[:, : KT * D], w_in[:, : KT * D])
                nc.sync.dma_start(w_sb[:, KT * D :], w_in[:, KT * D :])
            # Tails: columns [XH, T) of each k slice. Emitted before the
            # stores so the in-order sync FIFO can never park a
            # not-yet-ready store in front of pure loads.
            nc.sync.dma_start(x_st_v[:, :, XH:], xT_v[:, :, XH:])

            # ---- leaky relu on the head columns, one strided instruction
            # over all k slices (tail relus are emitted after the first B
            # drains; ACT is in-order)
            yT = persist.tile([128, KT * T], F16, tag="yT", name=f"yT_{rep}")
            yT_v = yT[:].rearrange("p (k t) -> p k t", k=KT)
            nc.scalar.activation(
                yT_v[:, :, :XH], x_st_v[:, :, :XH], AF.Prelu, alpha=NEG_SLOPE
            )

            # ---- A = leaky_relu(x_own) @ W1.T + bias in three M-groups
            # (128, 32, 64 rows). The own rows live in yT columns
            # [k*T, k*T + TI). The 32-row group re-covers rows 96..127 so
            # every E-matmul window starts at a legal base partition
            # (matmul operands must share base partition in {0, 32, 64}).
            a_parts = {}   # w -> (tile, offset)

            def emit_a(tag, rows, col0, windows):
                aps = psum.tile(
                    [rows, D], F32, tag="ps32", bufs=1,
                    padded_shape=[128, D], name=f"aps_{rep}_{tag}"
                )
                for k in range(KT):
                    nc.tensor.matmul(
                        aps[:],
                        yT[:, k * T + col0 : k * T + col0 + rows],
                        w1s(k),
                        start=(k == 0),
                        stop=False,
                    )
                nc.tensor.matmul(
                    aps[:], ones_sb[:1, :rows], bias_sb[:1, :],
                    start=False, stop=True,
                )
                # drain to fp16 so the E-matmul broadcast runs at 1 cyc/row
                aw = persist.tile(
                    [rows, D], F16, tag=f"a_{tag}", name=f"a_{rep}_{tag}"
                )
                nc.vector.tensor_copy(aw[:], aps[:])
                for w, off in windows:
                    a_parts[w] = (aw, off)

            out_v = out.rearrange("(i j p) c -> i p j c", i=LPC, p=128)
            abc = persist.tile([128, LPC * D], F32, tag="abc", name=f"abc_{rep}")
            n_grp = len(group_sizes)
            b_grp = [None] * n_grp

            def emit_bgroup(g):
                gsz = group_sizes[g]
                bg = persist.tile(
                    [128, gsz * D], F32, tag=f"b_grp{g}", name=f"b_grp{g}_{rep}"
                )
                for q in range(gsz):
                    jt = g_off[g] + q
                    bps = psum.tile(
                        [128, D], F32, tag="ps", bufs=3, name=f"bps_{rep}_{jt}"
                    )
                    for k in range(KT):
                        nc.tensor.matmul(
                            bps[:],
                            yT[:, k * T + 128 * jt : k * T + 128 * (jt + 1)],
                            w2s(k),
                            start=(k == 0),
                            stop=(k == KT - 1),
                        )
                    nc.scalar.activation(bg[:, q * D : (q + 1) * D], bps[:], AF.Copy)
                b_grp[g] = bg

            def emit_emm(il):
                """E-matmul broadcast of A rows for one il; returns the PSUM
                tile (caller drains it to abc)."""
                w, par = divmod(il, 2)
                src, off = a_parts[w]
                eps = psum.tile(
                    [128, D], F32, tag="eps", bufs=4, name=f"eps_{rep}_{il}"
                )
                nc.tensor.matmul(
                    eps[:],
                    ec_sb[par][off : off + 32],
                    src[off : off + 32, :],
                    start=True,
                    stop=True,
                )
                return eps

            def emit_abc(il):
                # drain on ACT, which only carries relus + B drains; the DVE
                # queue stays pure output adds
                eps = emit_emm(il)
                nc.scalar.activation(
                    abc[:, il * D : (il + 1) * D], eps[:], AF.Copy
                )

            def emit_add(il, g, a_slice=None):
                gsz = group_sizes[g]
                ot = outp.tile(
                    [128, gsz * D], F32, tag="ot", bufs=8,
                    name=f"ot_{rep}_{il}_{g}"
                )
                if a_slice is None:
                    a_slice = abc[:, il * D : (il + 1) * D]
                nc.vector.tensor_add(
                    ot[:].rearrange("p (j c) -> p j c", c=D),
                    b_grp[g][:].rearrange("p (j c) -> p j c", c=D),
                    a_slice.unsqueeze(1).broadcast_to((128, gsz, D)),
                )
                nc.sync.dma_start(
                    out_v[il, :, g_off[g] : g_off[g + 1], :],
                    ot[:].rearrange("p (j c) -> p j c", c=D),
                )

            ready_il = []
            ready_g = []

            def unlock_il(*ils):
                for il in ils:
                    emit_abc(il)
                for il in ils:
                    ready_il.append(il)
                    for g in ready_g:
                        emit_add(il, g)

            def unlock_g(g):
                emit_bgroup(g)
                ready_g.append(g)
                for il in ready_il:
                    emit_add(il, g)

            def emit_relu_tail(c0, c1):
                nc.scalar.activation(
                    yT_v[:, :, c0:c1], x_st_v[:, :, c0:c1],
                    AF.Prelu, alpha=NEG_SLOPE,
                )

            # windows: w0..2 live in the 128-row A group at offsets 0/32/64,
            # w3 in its own 32-row group, w4..5 in the 64-row group.
            a_specs = {
                0: ("g0", 128, 0, [(0, 0), (1, 32), (2, 64)]),
                3: ("g0b", 32, 96, [(3, 0)]),
                4: ("g1", 64, 128, [(4, 0), (5, 32)]),
            }
            # Head: A g0 chain, B g0, then E0/E1 whose PSUM feeds the first
            # two adds directly (skipping the abc-drain + semaphore level on
            # the first-store critical path); their abc drains go to Pool
            # immediately so the eps ring frees fast. All remaining A groups
            # and E broadcasts are emitted early - the PE is only ~20% busy,
            # and materializing all of abc up front decouples the later
            # j-groups from the A path entirely. The tail relus are split:
            # j-tiles 2..3 right after the first B drains so B g2/g3 aren't
            # parked behind the full relu stream on the in-order ACT queue.
            emit_a(*a_specs[0])
            emit_bgroup(0)
            ready_g.append(0)
            eps0 = emit_emm(0)
            eps1 = emit_emm(1)
            emit_add(0, 0, a_slice=eps0[:])
            emit_add(1, 0, a_slice=eps1[:])
            nc.vector.tensor_copy(abc[:, 0:D], eps0[:])
            nc.vector.tensor_copy(abc[:, D : 2 * D], eps1[:])
            ready_il.extend([0, 1])
            unlock_g(1)                      # B g1 + adds (0,1), (1,1)
            emit_relu_tail(XH, XH + 256)     # j-tiles 2,3 on ACT
            unlock_il(2, 3)
            unlock_g(2)
            emit_a(*a_specs[3])
            unlock_il(4, 5)
            unlock_g(3)
            emit_relu_tail(XH + 256, T)      # rest of the relus on ACT
            emit_a(*a_specs[4])
            unlock_il(6, 7)
            unlock_g(4)
            unlock_il(8, 9)
            unlock_il(10, 11)
            unlock_g(5)
            unlock_g(6)
            unlock_g(7)

    nc.compile()
    return nc


def _pack_kt(arr_t, nfree):
    """(D, nfree) k-major -> (128, KT*nfree) partition-packed SBUF layout."""
    return np.ascontiguousarray(
        arr_t.reshape(KT, 128, nfree).transpose(1, 0, 2).reshape(128, KT * nfree)
    )


def make_in_maps(x, W, bias):
    x = np.asarray(x, np.float32)
    W = np.asarray(W, np.float32)
    bias = np.asarray(bias, np.float32)
    x_t = np.ascontiguousarray(x.reshape(T, D).astype(np.float16))
    w_all = np.ascontiguousarray(
        np.ascontiguousarray(W.T.astype(np.float16))
        .reshape(2 * KT, 128, D)
        .transpose(1, 0, 2)
        .reshape(128, 2 * KT * D)
    )
    b2 = np.ascontiguousarray(bias.reshape(1, D).astype(np.float16))
    maps = []
    for r in range(NCORES):
        # rotate this core's own TI rows to the front of the t axis
        x_rot = np.roll(x_t, -r * TI, axis=0)
        xTr = _pack_kt(np.ascontiguousarray(x_rot.T), T)
        maps.append({"xT": xTr, "w_in": w_all, "bias": b2})
    return maps


_NC_CACHE = {}


def get_nc(repeats=1, group_sizes=(1, 1, 1, 1, 1, 2, 2, 3), gps_groups=()):
    key = (repeats, tuple(group_sizes), tuple(gps_groups))
    if key not in _NC_CACHE:
        _NC_CACHE[key] = build_nc(repeats=repeats, group_sizes=group_sizes, gps_groups=gps_groups)
    return _NC_CACHE[key]


def kernel(x, W, bias, group_sizes=(1, 1, 1, 1, 1, 2, 2, 3), gps_groups=()):
    nc = get_nc(1, group_sizes, gps_groups)
    maps = make_in_maps(x, W, bias)
    res = run_bass_kernel_spmd(nc, maps, list(range(NCORES)))
    outs = []
    for r in range(NCORES):
        o = res.results[r]["out"].reshape(LPC, T, D)
        outs.append(np.roll(o, r * TI, axis=1).reshape(LPC * T, D))
    return np.concatenate(outs, axis=0).reshape(L * L, Bdim, D)



# revision 2
# speedup vs baseline: 1.0044x; 1.0044x over previous
"""CatLayer Trainium2 kernel: transposed (channel-partition) layout,
fp16 output stores. TimelineSim: 63142 ns (baseline: 118670 ns).

Math: out[i,j,b,:] = W @ leaky_relu(concat(x[i,b,:], x[j,b,:])) + bias
    y  = leaky_relu(x)                    # (l, b, d)
    A  = y @ W[:, :d].T                   # (l, b, d)   (bias folded into B)
    B  = y @ W[:, d:].T + bias            # (l, b, d)
    out[i,j,b,:] = A[i,b,:] + B[j,b,:]

Sharding: i-rows of the (l x l) pair grid over 8 cores (12 rows each).

Everything on device is computed TRANSPOSED (partition = channel c):
    A^T[c, t_own]  = sum_k W1T[k, c] yT[k, t_own]
    B^T[c, t]      = sum_k W2T[k, c] yT[k, t]  + bias[c]
    out^T[c, (il, t)] = A^T[c, il*16 + t%16] + B^T[c, t]
The A term varies only with (c, b) so the DVE add reads it with a stride-0
broadcast over j; bias is per-partition here, so it rides the B drain's
activation (Identity + bias) for free. No partition-broadcast (E-matmul)
machinery at all.

The kernel is DMA-bound: all DMAs serialize on the shared DMA-engine pool
(~360 GB/s; fp16 out = 18.9 MB/core ~ 52.4 us, loads 2.6 MB ~ 7.3 us), so
the schedule keeps the DMA device saturated end-to-end:
  * loads stream in first-output dependency order: x chunk-0 (split
    k01/k23 so the first relu half starts earlier), then one contiguous
    DMA carrying [W2-ct0 | bias | W1-ct0] (everything the first B and A
    matmuls need), then the W tails and x chunks 1-2; the bias rides
    inside the W load so no extra DMA/HWDGE slot delays the stream;
  * the first add unit is small (3 il) so its store is staged on
    HWDGE/DGE while the last x load still occupies the DMA device, and
    grabs the device the moment it frees;
  * output is fp16 (rel err ~1e-3 vs the 2e-2 gate); host upcasts.

Engines:
    ACT: leaky-relus (per t-chunk), B^T PSUM->SBUF fp16 drains with the
         per-partition bias fused in (Identity+bias)
    PE : A^T/B^T matmuls (fp16 x fp16 -> fp32 PSUM), warmed up so the
         first real matmuls run at full clock
    DVE: A^T PSUM->SBUF copies + the output adds (fp16 2x mode)
    SP : all loads + all stores (in dependency order)
"""

import numpy as np
from contextlib import ExitStack

import concourse.bacc as bacc
import concourse.mybir as mybir
from concourse import tile
from concourse.bass_utils import run_bass_kernel_spmd

F32 = mybir.dt.float32
F16 = mybir.dt.float16
AF = mybir.ActivationFunctionType

L, Bdim, D = 96, 16, 512
NCORES = 8
LPC = L // NCORES          # 12 i-rows per core
T = L * Bdim               # 1536 (j,b) rows
KT = D // 128              # 4 k-tiles
TI = LPC * Bdim            # 192 own (i,b) rows
CT = 4                     # c-tiles of 128 channels
CHUNK = 512                # t-chunk size (one PSUM bank)
NCH = T // CHUNK           # 3 t-chunks
NEG_SLOPE = 0.1
W2OFF = KT * D             # w_sb column where the ct-major W2 block starts
BCOL = W2OFF + D           # bias bits live in cols [BCOL, BCOL+8)
WCOLS = 2 * KT * D + 8     # total w_in columns


def build_nc(n_warm=36, adds_per_unit=((4, 2, 2, 2), (1, 1, 1, 1), (1, 1, 1, 1))):
    """adds_per_unit[ch][ct] = number of add/store units for that (ct, chunk);
    each unit covers LPC//n consecutive il values."""
    nc = bacc.Bacc("TRN2", target_bir_lowering=False, debug=False)

    xT = nc.dram_tensor("xT", (128, KT * T), F16, kind="ExternalInput")
    w_in = nc.dram_tensor("w_in", (128, WCOLS), F16, kind="ExternalInput")
    # out[c, ch*LPC*CHUNK + il*CHUNK + t_in] = out^T[c, il, t = ch*CHUNK+t_in]
    out = nc.dram_tensor("out", (D, LPC * T), F16, kind="ExternalOutput")

    with tile.TileContext(nc) as tc, ExitStack() as ctx:
        persist = ctx.enter_context(tc.tile_pool(name="persist", bufs=1))
        bpool = ctx.enter_context(tc.tile_pool(name="bpool", bufs=1))
        psum = ctx.enter_context(tc.tile_pool(name="psum", bufs=7, space="PSUM"))
        outp = ctx.enter_context(tc.tile_pool(name="outp", bufs=1))

        ones_sb = persist.tile([1, 128], F16, tag="ones", name="ones_sb")
        nc.vector.memset(ones_sb[:], 1.0)

        # w_sb: cols [0, W2OFF) = W1T k-major (k*D + c);
        #       [W2OFF, BCOL) = W2T ct0 (k*128 + c_lo); [BCOL, BCOL+8) = bias
        #       bits (f32x4 as f16x8); [BCOL+8, ...) = W2T ct1-3 ct-major.
        w_sb = persist.tile([128, WCOLS], F16, tag="w", name="w_sb")
        biasT_sb = w_sb[:, BCOL : BCOL + 8].bitcast(F32)  # (128, CT) f32

        def w1s(k, ct):
            return w_sb[:, k * D + ct * 128 : k * D + (ct + 1) * 128]

        def w2s(k, ct):
            c0 = (W2OFF if ct == 0 else BCOL + 8 + (ct - 1) * D) + k * 128
            return w_sb[:, c0 : c0 + 128]

        # ---- PE warm-up: HAM runs the PE at reduced clock until it has seen
        # ~3us of continuous activity; keep it busy with narrow dummy matmuls
        # until the first real matmul (~4.7us) so the ramp is done by then.
        warm_ps = psum.tile([128, 128], F32, tag="warm", bufs=1, name="warm_ps")
        for _ in range(n_warm):
            nc.tensor.matmul(
                warm_ps[:], ones_sb[:1, :], ones_sb[:1, :], start=True, stop=True
            )

        # ---- input loads on the sync queue, in first-store dependency order
        x_st = persist.tile([128, KT * T], F16, tag="x_st", name="x_st")
        x_st_v = x_st[:].rearrange("p (k t) -> p k t", k=KT)
        xT_v = xT.ap().rearrange("p (k t) -> p k t", k=KT)
        nc.sync.dma_start(x_st_v[:, :2, :CHUNK], xT_v[:, :2, :CHUNK])
        nc.sync.dma_start(x_st_v[:, 2:, :CHUNK], xT_v[:, 2:, :CHUNK])
        nc.sync.dma_start(
            w_sb[:, W2OFF : BCOL + 8], w_in[:, W2OFF : BCOL + 8]
        )  # W2 ct0 + bias
        nc.sync.dma_start(w_sb[:, :W2OFF], w_in[:, :W2OFF])  # W1
        nc.sync.dma_start(w_sb[:, BCOL + 8 :], w_in[:, BCOL + 8 :])  # W2 ct1-3
        nc.sync.dma_start(
            x_st_v[:, :, CHUNK : 2 * CHUNK], xT_v[:, :, CHUNK : 2 * CHUNK]
        )
        nc.sync.dma_start(x_st_v[:, :, 2 * CHUNK :], xT_v[:, :, 2 * CHUNK :])

        yT = persist.tile([128, KT * T], F16, tag="yT", name="yT")
        yT_v = yT[:].rearrange("p (k t) -> p k t", k=KT)

        def relu(ks, ch):
            nc.scalar.activation(
                yT_v[:, ks, ch * CHUNK : (ch + 1) * CHUNK],
                x_st_v[:, ks, ch * CHUNK : (ch + 1) * CHUNK],
                AF.Prelu,
                alpha=NEG_SLOPE,
            )

        # A^T[c, t_own] per c-tile; drained to SBUF by DVE (plain copy)
        at_sb = persist.tile([128, CT * TI], F16, tag="at", name="at_sb")

        def emit_a_mm(ct):
            aps = psum.tile([128, TI], F32, tag="aps", bufs=3, name=f"aps_{ct}")
            for k in range(KT):
                nc.tensor.matmul(
                    aps[:],
                    w1s(k, ct),
                    yT[:, k * T : k * T + TI],
                    start=(k == 0),
                    stop=(k == KT - 1),
                )
            return aps

        def emit_a_copy(ct, aps):
            nc.vector.tensor_copy(at_sb[:, ct * TI : (ct + 1) * TI], aps[:])

        def emit_b(ct, ch):
            bps = psum.tile([128, CHUNK], F32, tag="bps", bufs=3, name=f"bps_{ct}_{ch}")
            for k in range(KT):
                nc.tensor.matmul(
                    bps[:],
                    w2s(k, ct),
                    yT[:, k * T + ch * CHUNK : k * T + (ch + 1) * CHUNK],
                    start=(k == 0),
                    stop=(k == KT - 1),
                )
            bsb = bpool.tile([128, CHUNK], F16, tag="bsb", bufs=6, name=f"bsb_{ct}_{ch}")
            # PSUM -> SBUF fp16 with the per-partition bias fused in
            nc.scalar.add(bsb[:], bps[:], biasT_sb[:, ct : ct + 1])
            return bsb

        NJ = CHUNK // Bdim  # 32 j values per chunk

        def emit_unit(ct, ch, bsb, il0, nil):
            """Add + store for il in [il0, il0+nil) of (ct, ch)."""
            ot = outp.tile([128, nil * CHUNK], F16, tag="ot", bufs=8,
                           name=f"ot_{ct}_{ch}_{il0}")
            a_sl = (
                at_sb[:, ct * TI + il0 * Bdim : ct * TI + (il0 + nil) * Bdim]
                .rearrange("p (il b) -> p il b", b=Bdim)
                .unsqueeze(2)
                .broadcast_to((128, nil, NJ, Bdim))
            )
            b_sl = (
                bsb[:]
                .rearrange("p (j b) -> p j b", b=Bdim)
                .unsqueeze(1)
                .broadcast_to((128, nil, NJ, Bdim))
            )
            nc.vector.tensor_add(
                ot[:].rearrange("p (il j b) -> p il j b", j=NJ, b=Bdim), b_sl, a_sl
            )
            col0 = ch * LPC * CHUNK + il0 * CHUNK
            nc.sync.dma_start(
                out.ap()[ct * 128 : (ct + 1) * 128, col0 : col0 + nil * CHUNK],
                ot[:],
            )

        def emit_units(ct, ch, bsb):
            nun = adds_per_unit[ch][ct]
            nil = LPC // nun
            for u in range(nun):
                emit_unit(ct, ch, bsb, u * nil, nil)

        # ---- schedule. B(0,0) first on PE (its inputs land earliest); the
        # A chain is gated on W1 and runs while W2's tail streams in.
        relu(slice(0, 2), 0)
        relu(slice(2, 4), 0)
        bsb00 = emit_b(0, 0)
        a_ps = [emit_a_mm(ct) for ct in range(CT)]
        emit_a_copy(0, a_ps[0])
        emit_units(0, 0, bsb00)
        emit_a_copy(1, a_ps[1])
        bsb = emit_b(1, 0)
        emit_units(1, 0, bsb)
        emit_a_copy(2, a_ps[2])
        relu(slice(0, 4), 1)
        bsb = emit_b(2, 0)
        emit_units(2, 0, bsb)
        emit_a_copy(3, a_ps[3])
        bsb = emit_b(3, 0)
        emit_units(3, 0, bsb)
        for ch in (1, 2):
            for ct in range(CT):
                bsb = emit_b(ct, ch)
                if ch == 1 and ct == 1:
                    relu(slice(0, 4), 2)
                emit_units(ct, ch, bsb)

    nc.compile()
    return nc


def _pack_kt(arr_t, nfree):
    """(D, nfree) k-major -> (128, KT*nfree) partition-packed SBUF layout."""
    return np.ascontiguousarray(
        arr_t.reshape(KT, 128, nfree).transpose(1, 0, 2).reshape(128, KT * nfree)
    )


def make_in_maps(x, W, bias):
    x = np.asarray(x, np.float32)
    W = np.asarray(W, np.float32)
    bias = np.asarray(bias, np.float32)
    x_t = np.ascontiguousarray(x.reshape(T, D).astype(np.float16))
    wt = np.ascontiguousarray(W.T.astype(np.float16))       # (2d, d) = (k_full, c)
    w1 = _pack_kt(wt[:D], D)                                # (128, KT*D) k-major
    # W2 ct-major: w2ct[p, ct, k*128 + c_lo] = W2T[128k+p, ct*128+c_lo]
    w2ct = wt[D:].reshape(KT, 128, CT, 128).transpose(1, 2, 0, 3)  # (128,CT,KT,128)
    w2 = w2ct.reshape(128, KT * D)
    bias_bits = (
        bias.reshape(CT, 128).T.astype(np.float32).copy().view(np.float16)
    )  # (128, 8)
    w_all = np.ascontiguousarray(
        np.concatenate(
            [w1, w2[:, :D], bias_bits, w2[:, D:]], axis=1, dtype=np.float16
        )
    )
    maps = []
    for r in range(NCORES):
        # rotate this core's own TI rows to the front of the t axis
        x_rot = np.roll(x_t, -r * TI, axis=0)
        xTr = _pack_kt(np.ascontiguousarray(x_rot.T), T)
        maps.append({"xT": xTr, "w_in": w_all})
    return maps


_NC_CACHE = {}


def get_nc(repeats=1, **kw):
    key = (repeats, tuple(sorted(kw.items())))
    if key not in _NC_CACHE:
        _NC_CACHE[key] = build_nc(**kw)
    return _NC_CACHE[key]


def kernel(x, W, bias, **kw):
    nc = get_nc(1, **kw)
    maps = make_in_maps(x, W, bias)
    res = run_bass_kernel_spmd(nc, maps, list(range(NCORES)))
    outs = []
    for r in range(NCORES):
        o = res.results[r]["out"].reshape(D, NCH, LPC, CHUNK)
        arr = o.transpose(2, 1, 3, 0).reshape(LPC, T, D)
        outs.append(np.roll(arr, r * TI, axis=1))
    full = np.concatenate(outs, axis=0).astype(np.float32)
    return np.ascontiguousarray(full.reshape(L * L, Bdim, D))


# revision 3
# speedup vs baseline: 1.0176x; 1.0131x over previous
"""CatLayer Trainium2 kernel, b-sharded: each core owns 2 of the 16 batch
rows, so it loads only its x slice (196KB vs 1.57MB) -- both i and j of the
pair grid span all l locally. Transposed (channel-partition) compute,
fp16 output stores.

Math: out[i,j,b,:] = W @ leaky_relu(concat(x[i,b,:], x[j,b,:])) + bias
    y  = leaky_relu(x_own)                # (l, 2, d) -> t' = i*2+b', 192 rows
    A^T[c, t'] = sum_k W1T[k, c] yT[k, t']
    B^T[c, t'] = sum_k W2T[k, c] yT[k, t'] + bias[c]
    out^T[c, (i, j, b')] = A^T[c, i*2+b'] + B^T[c, j*2+b']

The kernel is DMA-bound (shared 360 GB/s DMA device; fp16 out 18.9 MB/core
= 52.4 us, loads now only 1.25 MB = 3.5 us). Loads stream [x | W2ct0+bias+
W1ct0 | W1rest | W2rest]; the whole A/B compute is tiny (8 matmuls of 192
free per c-tile) so the first store chain (x-sem, relu, matmuls, drains,
first small add) limits the start; after that the store stream runs
gap-free to the end.

Engines:
    ACT: the two leaky-relu halves, B^T drains with per-partition bias
         fused (Identity+bias), late A^T drains
    PE : A^T/B^T matmuls (fp16 x fp16 -> fp32 PSUM), warmed up
    DVE: ct0's A^T drain + all output adds (fp16 2x mode)
    SP : all loads + all stores
"""

import numpy as np
from contextlib import ExitStack

import concourse.bacc as bacc
import concourse.mybir as mybir
from concourse import tile
from concourse.bass_utils import run_bass_kernel_spmd

F32 = mybir.dt.float32
F16 = mybir.dt.float16
AF = mybir.ActivationFunctionType

L, Bdim, D = 96, 16, 512
NCORES = 8
BPC = Bdim // NCORES       # 2 batch rows per core
T = L * BPC                # 192 (i, b') rows of this core's y
KT = D // 128              # 4 k-tiles
CT = 4                     # c-tiles of 128 channels
NEG_SLOPE = 0.1
# w_sb/w_in column layout (same spirit as the i-sharded kernel):
#   [0, D) W2ct0 | [D, D+32) bias+pad | [D+32, 2D+32) W1ct0
#   | [2D+32, 5D+32) W1 ct1-3 | [5D+32, 8D+32) W2 ct1-3
BCOL = D
W1CT0 = D + 32
W1REST = W1CT0 + D
W2REST = W1REST + 3 * D
WCOLS = W2REST + 3 * D

DEF_UNITS = (
    (2, 2, 4, 8, 8, 8, 16, 16, 16, 16),
    (4, 12, 16, 16, 16, 16, 16),
    (16, 16, 16, 16, 16, 16),
    (16, 16, 16, 16, 16, 16),
)


def build_nc(n_warm=32, units=DEF_UNITS, n_psum_units=0):
    """units[ct] = tuple of i-range sizes for that c-tile's add/store units
    (each >= 2 to keep DRAM runs >= 512B); sizes must sum to L."""
    nc = bacc.Bacc("TRN2", target_bir_lowering=False, debug=False)

    xT = nc.dram_tensor("xT", (128, KT * T), F16, kind="ExternalInput")
    w_in = nc.dram_tensor("w_in", (128, WCOLS), F16, kind="ExternalInput")
    # out[c, i*192 + j*2 + b'] = out^T[c, i, j, b']
    out = nc.dram_tensor("out", (D, L * T), F16, kind="ExternalOutput")

    with tile.TileContext(nc) as tc, ExitStack() as ctx:
        persist = ctx.enter_context(tc.tile_pool(name="persist", bufs=1))
        psum = ctx.enter_context(tc.tile_pool(name="psum", bufs=5, space="PSUM"))
        outp = ctx.enter_context(tc.tile_pool(name="outp", bufs=1))

        ones_sb = persist.tile([1, 128], F16, tag="ones", name="ones_sb")
        nc.vector.memset(ones_sb[:], 1.0)

        w_sb = persist.tile([128, WCOLS], F16, tag="w", name="w_sb")
        biasT_sb = w_sb[:, BCOL : BCOL + CT]
        # fp32 copy of the bias for DVE tensor_scalar_add (requires f32 scalar)
        bias32 = persist.tile([128, CT], F32, tag="b32", name="bias32")

        def w1s(k, ct):
            c0 = (W1CT0 if ct == 0 else W1REST + (ct - 1) * D) + k * 128
            return w_sb[:, c0 : c0 + 128]

        def w2s(k, ct):
            c0 = (0 if ct == 0 else W2REST + (ct - 1) * D) + k * 128
            return w_sb[:, c0 : c0 + 128]

        warm_ps = psum.tile([128, 128], F32, tag="warm", bufs=1, name="warm_ps")
        for _ in range(n_warm):
            nc.tensor.matmul(
                warm_ps[:], ones_sb[:1, :], ones_sb[:1, :], start=True, stop=True
            )

        # ---- loads: x first (gates the relu), then the W block the first
        # matmuls need, then the W tails.
        x_st = persist.tile([128, KT * T], F16, tag="x_st", name="x_st")
        nc.sync.dma_start(x_st[:], xT.ap())
        nc.sync.dma_start(w_sb[:, :W1REST], w_in[:, :W1REST])
        nc.sync.dma_start(w_sb[:, W1REST:W2REST], w_in[:, W1REST:W2REST])
        nc.sync.dma_start(w_sb[:, W2REST:], w_in[:, W2REST:])

        yT = persist.tile([128, KT * T], F16, tag="yT", name="yT")
        # relu halves split ACROSS ENGINES (k01 on ACT, k23 on DVE as
        # (x*0.1) max x) so both finish ~0.5us after the x DMA sem and the
        # scheduler never mispredicts the ACT queue (act-table load) into
        # head-blocking ct0's matmuls behind W-tail-gated ones.
        nc.scalar.activation(
            yT[:, : 2 * T], x_st[:, : 2 * T], AF.Prelu, alpha=NEG_SLOPE
        )
        nc.vector.tensor_copy(bias32[:], biasT_sb)
        nc.vector.scalar_tensor_tensor(
            yT[:, 2 * T :],
            x_st[:, 2 * T :],
            NEG_SLOPE,
            x_st[:, 2 * T :],
            mybir.AluOpType.mult,
            mybir.AluOpType.max,
        )

        # A^T and B^T share one PSUM bank per c-tile: cols [0,192) = A,
        # [192, 384) = B.
        ab_sb = persist.tile([128, CT * 2 * T], F16, tag="ab", name="ab_sb")

        def a_sl_of(ct):
            return ab_sb[:, ct * 2 * T : ct * 2 * T + T]

        def b_sl_of(ct):
            return ab_sb[:, ct * 2 * T + T : (ct + 1) * 2 * T]

        def emit_ab(ct):
            # B first: its drain is the first gate of the ct's add stream
            ps = psum.tile([128, 2 * T], F32, tag="abps", bufs=4, name=f"abps_{ct}")
            for k in range(KT):
                nc.tensor.matmul(
                    ps[:, T:], w2s(k, ct), yT[:, k * T : (k + 1) * T],
                    start=(k == 0), stop=(k == KT - 1),
                )
            for k in range(KT):
                nc.tensor.matmul(
                    ps[:, :T], w1s(k, ct), yT[:, k * T : (k + 1) * T],
                    start=(k == 0), stop=(k == KT - 1),
                )
            # ct0: both drains on DVE, the same queue as the adds -- no
            # cross-engine sem hop on the first-store critical path; the
            # bias rides tensor_scalar_add. ct1-3: drains on ACT (idle).
            if ct == 0:
                nc.vector.tensor_scalar_add(
                    b_sl_of(ct), ps[:, T:], bias32[:, ct : ct + 1]
                )
                nc.vector.tensor_copy(a_sl_of(ct), ps[:, :T])
            else:
                nc.scalar.add(b_sl_of(ct), ps[:, T:], biasT_sb[:, ct : ct + 1])
                nc.scalar.activation(a_sl_of(ct), ps[:, :T], AF.Copy)
            return ps

        def emit_unit(ct, i0, ni, ps=None):
            ot = outp.tile([128, ni * T], F16, tag="ot", bufs=8,
                           name=f"ot_{ct}_{i0}")
            if ps is not None:
                # first unit(s): add straight from the A/B PSUM halves,
                # skipping both drains on the first-store critical path
                a_src, b_src = ps[:, :T], ps[:, T:]
            else:
                a_src, b_src = a_sl_of(ct), b_sl_of(ct)
            a_sl = (
                a_src[:, i0 * BPC : (i0 + ni) * BPC]
                .rearrange("p (i b) -> p i b", b=BPC)
                .unsqueeze(2)
                .broadcast_to((128, ni, L, BPC))
            )
            b_sl = (
                b_src
                .rearrange("p (j b) -> p j b", b=BPC)
                .unsqueeze(1)
                .broadcast_to((128, ni, L, BPC))
            )
            nc.vector.tensor_add(
                ot[:].rearrange("p (i j b) -> p i j b", j=L, b=BPC), b_sl, a_sl
            )
            nc.sync.dma_start(
                out.ap()[ct * 128 : (ct + 1) * 128, i0 * T : (i0 + ni) * T],
                ot[:],
            )

        for ct in range(CT):
            ps = emit_ab(ct)
            i0 = 0
            for u, ni in enumerate(units[ct]):
                emit_unit(ct, i0, ni,
                          ps if (ct == 0 and u < n_psum_units) else None)
                i0 += ni
            assert i0 == L

    nc.compile()
    return nc


def _pack_kt(arr_t, nfree):
    """(D, nfree) k-major -> (128, KT*nfree) partition-packed SBUF layout."""
    return np.ascontiguousarray(
        arr_t.reshape(KT, 128, nfree).transpose(1, 0, 2).reshape(128, KT * nfree)
    )


def make_in_maps(x, W, bias):
    x = np.asarray(x, np.float32)
    W = np.asarray(W, np.float32)
    bias = np.asarray(bias, np.float32)
    wt = np.ascontiguousarray(W.T.astype(np.float16))       # (2d, d) = (k_full, c)
    w1 = wt[:D].reshape(KT, 128, CT, 128).transpose(1, 2, 0, 3).reshape(128, KT * D)
    w2 = wt[D:].reshape(KT, 128, CT, 128).transpose(1, 2, 0, 3).reshape(128, KT * D)
    bias_blk = np.zeros((128, 32), np.float16)
    bias_blk[:, :CT] = bias.reshape(CT, 128).T.astype(np.float16)
    w_all = np.ascontiguousarray(
        np.concatenate(
            [w2[:, :D], bias_blk, w1, w2[:, D:]], axis=1, dtype=np.float16
        )
    )
    maps = []
    for r in range(NCORES):
        xs = x[:, r * BPC : (r + 1) * BPC, :].reshape(T, D).astype(np.float16)
        xTr = _pack_kt(np.ascontiguousarray(xs.T), T)
        maps.append({"xT": xTr, "w_in": w_all})
    return maps


_NC_CACHE = {}


def get_nc(repeats=1, **kw):
    key = (repeats, tuple(sorted(kw.items())))
    if key not in _NC_CACHE:
        _NC_CACHE[key] = build_nc(**kw)
    return _NC_CACHE[key]


def kernel(x, W, bias, **kw):
    nc = get_nc(1, **kw)
    maps = make_in_maps(x, W, bias)
    res = run_bass_kernel_spmd(nc, maps, list(range(NCORES)))
    outs = []
    for r in range(NCORES):
        o = res.results[r]["out"].reshape(D, L, L, BPC)
        outs.append(o.transpose(1, 2, 3, 0))    # (i, j, b', c)
    full = np.concatenate(outs, axis=2).astype(np.float32)
    return np.ascontiguousarray(full.reshape(L * L, Bdim, D))


# revision 4
# speedup vs baseline: 1.0263x; 1.0086x over previous
"""CatLayer Trainium2 kernel, b-sharded: each core owns 2 of the 16 batch
rows, so it loads only its x slice (196KB vs 1.57MB) -- both i and j of the
pair grid span all l locally. Transposed (channel-partition) compute,
fp16 output stores.

Math: out[i,j,b,:] = W @ leaky_relu(concat(x[i,b,:], x[j,b,:])) + bias
    y  = leaky_relu(x_own)                # (l, 2, d) -> t' = i*2+b', 192 rows
    A^T[c, t'] = sum_k W1T[k, c] yT[k, t']
    B^T[c, t'] = sum_k W2T[k, c] yT[k, t'] + bias[c]
    out^T[c, (i, j, b')] = A^T[c, i*2+b'] + B^T[c, j*2+b']

The kernel is DMA-bound (shared 360 GB/s DMA device; fp16 out 18.9 MB/core
= 52.4 us, loads now only 1.25 MB = 3.5 us). Loads stream [x | W2ct0+bias+
W1ct0 | W1rest | W2rest]; the whole A/B compute is tiny (8 matmuls of 192
free per c-tile) so the first store chain (x-sem, relu, matmuls, drains,
first small add) limits the start; after that the store stream runs
gap-free to the end.

Engines:
    ACT: the two leaky-relu halves, B^T drains with per-partition bias
         fused (Identity+bias), late A^T drains
    PE : A^T/B^T matmuls (fp16 x fp16 -> fp32 PSUM), warmed up
    DVE: ct0's A^T drain + all output adds (fp16 2x mode)
    SP : all loads + all stores
"""

import numpy as np
from contextlib import ExitStack

import concourse.bacc as bacc
import concourse.mybir as mybir
from concourse import tile
from concourse.bass_utils import run_bass_kernel_spmd

F32 = mybir.dt.float32
F16 = mybir.dt.float16
AF = mybir.ActivationFunctionType

L, Bdim, D = 96, 16, 512
NCORES = 8
BPC = Bdim // NCORES       # 2 batch rows per core
T = L * BPC                # 192 (i, b') rows of this core's y
KT = D // 128              # 4 k-tiles
CT = 4                     # c-tiles of 128 channels
NEG_SLOPE = 0.1
# w_sb/w_in column layout (same spirit as the i-sharded kernel):
#   [0, D) W2ct0 | [D, D+32) bias+pad | [D+32, 2D+32) W1ct0
#   | [2D+32, 5D+32) W1 ct1-3 | [5D+32, 8D+32) W2 ct1-3
BCOL = D
W1CT0 = D + 32
W1REST = W1CT0 + D
W2REST = W1REST + 3 * D
WCOLS = W2REST + 3 * D

DEF_UNITS = (
    (3, 6, 6, 8, 8, 8, 16, 16, 16, 9),
    (16, 16, 16, 16, 16, 16),
    (16, 16, 16, 16, 16, 16),
    (16, 16, 16, 16, 16, 16),
)


def build_nc(n_warm=26, units=DEF_UNITS, n_psum_units=0):
    """units[ct] = tuple of i-range sizes for that c-tile's add/store units
    (each >= 2 to keep DRAM runs >= 512B); sizes must sum to L."""
    nc = bacc.Bacc("TRN2", target_bir_lowering=False, debug=False)

    xT = nc.dram_tensor("xT", (128, KT * T), F16, kind="ExternalInput")
    w_in = nc.dram_tensor("w_in", (128, WCOLS), F16, kind="ExternalInput")
    # out[c, i*192 + j*2 + b'] = out^T[c, i, j, b']
    out = nc.dram_tensor("out", (D, L * T), F16, kind="ExternalOutput")

    with tile.TileContext(nc) as tc, ExitStack() as ctx:
        persist = ctx.enter_context(tc.tile_pool(name="persist", bufs=1))
        psum = ctx.enter_context(tc.tile_pool(name="psum", bufs=5, space="PSUM"))
        outp = ctx.enter_context(tc.tile_pool(name="outp", bufs=1))

        ones_sb = persist.tile([1, 128], F16, tag="ones", name="ones_sb")
        nc.vector.memset(ones_sb[:], 1.0)

        w_sb = persist.tile([128, WCOLS], F16, tag="w", name="w_sb")
        biasT_sb = w_sb[:, BCOL : BCOL + CT]
        # fp32 copy of the bias for DVE tensor_scalar_add (requires f32 scalar)
        bias32 = persist.tile([128, CT], F32, tag="b32", name="bias32")

        def w1s(k, ct):
            c0 = (W1CT0 if ct == 0 else W1REST + (ct - 1) * D) + k * 128
            return w_sb[:, c0 : c0 + 128]

        def w2s(k, ct):
            c0 = (0 if ct == 0 else W2REST + (ct - 1) * D) + k * 128
            return w_sb[:, c0 : c0 + 128]

        warm_ps = psum.tile([128, 128], F32, tag="warm", bufs=1, name="warm_ps")
        for _ in range(n_warm):
            nc.tensor.matmul(
                warm_ps[:], ones_sb[:1, :], ones_sb[:1, :], start=True, stop=True
            )

        # ---- loads: x first (gates the relu), then the W block the first
        # matmuls need, then the W tails.
        x_st = persist.tile([128, KT * T], F16, tag="x_st", name="x_st")
        nc.sync.dma_start(x_st[:], xT.ap())
        nc.sync.dma_start(w_sb[:, :W1REST], w_in[:, :W1REST])
        nc.sync.dma_start(w_sb[:, W1REST:W2REST], w_in[:, W1REST:W2REST])
        nc.sync.dma_start(w_sb[:, W2REST:], w_in[:, W2REST:])

        yT = persist.tile([128, KT * T], F16, tag="yT", name="yT")
        # relu halves split ACROSS ENGINES (k01 on ACT, k23 on DVE as
        # (x*0.1) max x) so both finish ~0.5us after the x DMA sem and the
        # scheduler never mispredicts the ACT queue (act-table load) into
        # head-blocking ct0's matmuls behind W-tail-gated ones.
        nc.scalar.activation(
            yT[:, : 2 * T], x_st[:, : 2 * T], AF.Prelu, alpha=NEG_SLOPE
        )
        nc.vector.tensor_copy(bias32[:], biasT_sb)
        nc.vector.scalar_tensor_tensor(
            yT[:, 2 * T :],
            x_st[:, 2 * T :],
            NEG_SLOPE,
            x_st[:, 2 * T :],
            mybir.AluOpType.mult,
            mybir.AluOpType.max,
        )

        # A^T and B^T share one PSUM bank per c-tile: cols [0,192) = A,
        # [192, 384) = B.
        ab_sb = persist.tile([128, CT * 2 * T], F16, tag="ab", name="ab_sb")

        def a_sl_of(ct):
            return ab_sb[:, ct * 2 * T : ct * 2 * T + T]

        def b_sl_of(ct):
            return ab_sb[:, ct * 2 * T + T : (ct + 1) * 2 * T]

        def emit_ab(ct):
            # B first: its drain is the first gate of the ct's add stream
            ps = psum.tile([128, 2 * T], F32, tag="abps", bufs=4, name=f"abps_{ct}")
            for k in range(KT):
                nc.tensor.matmul(
                    ps[:, T:], w2s(k, ct), yT[:, k * T : (k + 1) * T],
                    start=(k == 0), stop=(k == KT - 1),
                )
            for k in range(KT):
                nc.tensor.matmul(
                    ps[:, :T], w1s(k, ct), yT[:, k * T : (k + 1) * T],
                    start=(k == 0), stop=(k == KT - 1),
                )
            # ct0: both drains on DVE, the same queue as the adds -- no
            # cross-engine sem hop on the first-store critical path; the
            # bias rides tensor_scalar_add. ct1-3: drains on ACT (idle).
            if ct == 0:
                nc.vector.tensor_scalar_add(
                    b_sl_of(ct), ps[:, T:], bias32[:, ct : ct + 1]
                )
                nc.vector.tensor_copy(a_sl_of(ct), ps[:, :T])
            else:
                nc.scalar.add(b_sl_of(ct), ps[:, T:], biasT_sb[:, ct : ct + 1])
                nc.scalar.activation(a_sl_of(ct), ps[:, :T], AF.Copy)
            return ps

        def emit_unit(ct, i0, ni, ps=None):
            ot = outp.tile([128, ni * T], F16, tag="ot", bufs=8,
                           name=f"ot_{ct}_{i0}")
            if ps is not None:
                # first unit(s): add straight from the A/B PSUM halves,
                # skipping both drains on the first-store critical path
                a_src, b_src = ps[:, :T], ps[:, T:]
            else:
                a_src, b_src = a_sl_of(ct), b_sl_of(ct)
            a_sl = (
                a_src[:, i0 * BPC : (i0 + ni) * BPC]
                .rearrange("p (i b) -> p i b", b=BPC)
                .unsqueeze(2)
                .broadcast_to((128, ni, L, BPC))
            )
            b_sl = (
                b_src
                .rearrange("p (j b) -> p j b", b=BPC)
                .unsqueeze(1)
                .broadcast_to((128, ni, L, BPC))
            )
            nc.vector.tensor_add(
                ot[:].rearrange("p (i j b) -> p i j b", j=L, b=BPC), b_sl, a_sl
            )
            nc.sync.dma_start(
                out.ap()[ct * 128 : (ct + 1) * 128, i0 * T : (i0 + ni) * T],
                ot[:],
            )

        for ct in range(CT):
            ps = emit_ab(ct)
            i0 = 0
            for u, ni in enumerate(units[ct]):
                emit_unit(ct, i0, ni,
                          ps if (ct == 0 and u < n_psum_units) else None)
                i0 += ni
            assert i0 == L

    nc.compile()
    return nc


def _pack_kt(arr_t, nfree):
    """(D, nfree) k-major -> (128, KT*nfree) partition-packed SBUF layout."""
    return np.ascontiguousarray(
        arr_t.reshape(KT, 128, nfree).transpose(1, 0, 2).reshape(128, KT * nfree)
    )


def make_in_maps(x, W, bias):
    x = np.asarray(x, np.float32)
    W = np.asarray(W, np.float32)
    bias = np.asarray(bias, np.float32)
    wt = np.ascontiguousarray(W.T.astype(np.float16))       # (2d, d) = (k_full, c)
    w1 = wt[:D].reshape(KT, 128, CT, 128).transpose(1, 2, 0, 3).reshape(128, KT * D)
    w2 = wt[D:].reshape(KT, 128, CT, 128).transpose(1, 2, 0, 3).reshape(128, KT * D)
    bias_blk = np.zeros((128, 32), np.float16)
    bias_blk[:, :CT] = bias.reshape(CT, 128).T.astype(np.float16)
    w_all = np.ascontiguousarray(
        np.concatenate(
            [w2[:, :D], bias_blk, w1, w2[:, D:]], axis=1, dtype=np.float16
        )
    )
    maps = []
    for r in range(NCORES):
        xs = x[:, r * BPC : (r + 1) * BPC, :].reshape(T, D).astype(np.float16)
        xTr = _pack_kt(np.ascontiguousarray(xs.T), T)
        maps.append({"xT": xTr, "w_in": w_all})
    return maps


_NC_CACHE = {}


def get_nc(repeats=1, **kw):
    key = (repeats, tuple(sorted(kw.items())))
    if key not in _NC_CACHE:
        _NC_CACHE[key] = build_nc(**kw)
    return _NC_CACHE[key]


def kernel(x, W, bias, **kw):
    nc = get_nc(1, **kw)
    maps = make_in_maps(x, W, bias)
    res = run_bass_kernel_spmd(nc, maps, list(range(NCORES)))
    outs = []
    for r in range(NCORES):
        o = res.results[r]["out"].reshape(D, L, L, BPC)
        outs.append(o.transpose(1, 2, 3, 0))    # (i, j, b', c)
    full = np.concatenate(outs, axis=2).astype(np.float32)
    return np.ascontiguousarray(full.reshape(L * L, Bdim, D))
